# revision 12
# baseline (speedup 1.0000x reference)
"""Trainium2 Bass kernel for a BinaryNet conv block.

Pipeline (per core, data-parallel over batch):
  sign(x) -> conv3x3(sign(w1)) -> BN1 -> sign -> conv3x3(sign(w2))
          -> maxpool2x2 -> BN2

Implementation notes:
  - Activations are +-0.5, weights +-1.0 in fp8e4 (exactly representable);
    convs run as 9 shifted-window matmuls with DoubleRow perf mode (K=256
    contraction per instruction), accumulating exactly into fp32 PSUM.
  - BN1+sign is fused into one ScalarE Sign activation against a
    host-precomputed per-channel threshold. Conv outputs are exact
    integers, so an integer cutoff k_c reproduces the reference's fp32
    sign decisions bit-exactly.
  - Spatial layout is channel-major [ci, y*(W+2)+x] with a zero border so
    the 9 taps are just constant AP offsets.
  - The bass2jax/pseudo-DMA path allows only ONE sync wait per DMA and has
    8 DMA lanes, so the kernel uses exactly 6 DMAs (1 packed consts, 4 x
    loads into DISTINCT tiles, 1 y store); no DMA destination is ever
    reused, so every DMA needs at most one semaphore wait.
"""

import os
import numpy as np

os.environ.setdefault("MYCRO_LOCAL_CACHE", "1")

N_CORES = 8
C = 256
NCHUNK = 2  # channel chunks of 128
KP = 128

# packed consts layout (bytes per partition)
W1_OFF = 0
W2_OFF = 4608
ID8_OFF = 9216  # bf16 identity, 256 B
ID32_OFF = 9472  # f32 identity, 512 B
NT1_OFF = 9984  # f32 [2]
S2_OFF = 9992
B2_OFF = 10000
CONST_B = 10016


def build_program(B, H, W, psum_stretch=1536):
    """Build the per-core Bass program. B images of HxWxC per core."""
    import concourse.bass as bass
    import concourse.bacc as bacc
    import concourse.tile as tile
    from concourse import mybir

    F32 = mybir.dt.float32
    FP8 = mybir.dt.float8e4
    BF16 = mybir.dt.bfloat16
    U8 = mybir.dt.uint8
    DR = mybir.MatmulPerfMode.DoubleRow
    Alu = mybir.AluOpType
    Act = mybir.ActivationFunctionType

    Hp, Wp = H + 2, W + 2
    S_pad = Hp * Wp
    DOFF = 32  # left zero pad inside each channel-chunk row buffer
    S_chunk = ((S_pad + DOFF + 32 + 15) // 16) * 16  # right pad >= 32
    RB = 2 * W  # transpose block = 2 image rows
    assert RB <= 128
    NB = H // 2  # transpose blocks per image
    G = 7 if NB % 7 == 0 else (2 if NB % 2 == 0 else 1)  # blocks per psum group
    NG = NB // G
    PO = (H // 2) * (W // 2)
    OB = min(112, PO)  # output transpose block (partitions)
    assert PO % OB == 0
    NOB = PO // OB

    def split_stretch(total, step):
        out, a = [], 0
        while a < total:
            out.append((a, min(step, total - a)))
            a += step
        return out

    conv1_st = [(Wp + a, n) for a, n in split_stretch(H * Wp, psum_stretch)]
    max_rows = (psum_stretch // Wp) // 2 * 2
    row_groups = []
    r = 0
    while r < H:
        g = min(max_rows, H - r)
        row_groups.append((r, g))
        r += g
    conv2_st = [((1 + r0) * Wp, rg * Wp, r0, rg) for r0, rg in row_groups]
    PS_COLS = psum_stretch

    nc = bacc.Bacc("TRN2", target_bir_lowering=False, debug=False)

    x_h = nc.dram_tensor("x", [B, H * W, C], F32, kind="ExternalInput")
    cb_h = nc.dram_tensor("cb", [KP, CONST_B], U8, kind="ExternalInput")
    y_h = nc.dram_tensor("y", [B, PO, C], F32, kind="ExternalOutput")

    def dram_ap(handle, offset, dims):
        return bass.AP(
            tensor=handle.ap().tensor, offset=offset, ap=[list(d) for d in dims]
        )

    with tile.TileContext(nc) as tc:
        from contextlib import ExitStack

        with ExitStack() as ctx:
            consts = ctx.enter_context(tc.tile_pool(name="consts", bufs=1))
            xnat_p = ctx.enter_context(tc.tile_pool(name="xnat", bufs=1))
            xsg_p = ctx.enter_context(tc.tile_pool(name="xsg", bufs=2))
            xsT_p = ctx.enter_context(tc.tile_pool(name="xsT", bufs=2))
            hsT_p = ctx.enter_context(tc.tile_pool(name="hsT", bufs=2))
            pr_p = ctx.enter_context(tc.tile_pool(name="prp", bufs=2))
            po_p = ctx.enter_context(tc.tile_pool(name="pop", bufs=2))
            onat_p = ctx.enter_context(tc.tile_pool(name="onat", bufs=1))
            convp = ctx.enter_context(tc.tile_pool(name="convp", bufs=2, space="PSUM"))
            tp_p = ctx.enter_context(tc.tile_pool(name="tpp", bufs=2, space="PSUM"))

            # --- packed constants: one DMA, bitcast views ---
            cb = consts.tile([KP, CONST_B], U8)
            nc.sync.dma_start(out=cb, in_=cb_h.ap())
            w1sb = cb[:, W1_OFF : W1_OFF + 4608].bitcast(FP8).rearrange(
                "p (t j k m) -> p t j k m", t=9, j=NCHUNK, k=2
            )
            w2sb = cb[:, W2_OFF : W2_OFF + 4608].bitcast(FP8).rearrange(
                "p (t j k m) -> p t j k m", t=9, j=NCHUNK, k=2
            )
            id8sb = cb[:, ID8_OFF : ID8_OFF + 256].bitcast(BF16)
            id32sb = cb[:, ID32_OFF : ID32_OFF + 512].bitcast(F32)
            nt1sb = cb[:, NT1_OFF : NT1_OFF + 8].bitcast(F32)
            s2sb = cb[:, S2_OFF : S2_OFF + 8].bitcast(F32)
            b2sb = cb[:, B2_OFF : B2_OFF + 8].bitcast(F32)

            def border_memsets(buf):
                # rows 0 and H+1 plus left/right pads
                nc.vector.memset(buf[:, :, 0 : DOFF + Wp], 0.0)
                nc.vector.memset(buf[:, :, DOFF + (H + 1) * Wp : S_chunk], 0.0)

            def col_border_memset(buf):
                rows = buf[:, :, DOFF + Wp : DOFF + (H + 1) * Wp].rearrange(
                    "p j (r w) -> p j r w", w=Wp
                )
                nc.vector.memset(rows[:, :, :, 0 :: (W + 1)], 0.0)

            def conv(inbuf, wsb, stretches, psum_tiles_cb):
                for si, st in enumerate(stretches):
                    cs, cn = st[0], st[1]
                    for j in range(NCHUNK):
                        ps = convp.tile([KP, PS_COLS], F32, tag="cv", name=f"cv{si}{j}")
                        for t in range(9):
                            dy, dx = t // 3, t % 3
                            off = (dy - 1) * Wp + (dx - 1)
                            lhsT = wsb[:, t, j]
                            for c0 in range(0, cn, 512):
                                n = min(512, cn - c0)
                                a = DOFF + cs + off + c0
                                rhs = inbuf[:, :, a : a + n]
                                nc.tensor.matmul(
                                    ps[:, c0 : c0 + n],
                                    lhsT,
                                    rhs,
                                    start=(t == 0),
                                    stop=(t == 8),
                                    perf_mode=DR,
                                )
                        psum_tiles_cb(si, j, ps, st)

            HB = min(2, B)  # images per output DMA
            onat = None

            for img in range(B):
                # ---------- input: one DMA + sign + transpose to channel-major
                # distinct destination tile per image: DMA dests are never
                # reused, so each x-load carries zero semaphore waits.
                xn = xnat_p.tile([RB, NB, C], F32, tag=f"xn{img}", name=f"xn{img}")
                nc.sync.dma_start(
                    out=xn,
                    in_=dram_ap(x_h, img * H * W * C, [[C, RB], [RB * C, NB], [1, C]]),
                )

                xsT = xsT_p.tile(
                    [KP, NCHUNK, S_chunk], FP8, tag="xsT", name=f"xsT{img}"
                )
                border_memsets(xsT)
                col_border_memset(xsT)
                for g in range(NG):
                    xg = xsg_p.tile([RB, G, C], BF16, tag="xg", name=f"xg{img}{g}")
                    nc.vector.tensor_scalar(
                        xg, xn[:, g * G : (g + 1) * G, :], 0.0, 0.5,
                        Alu.is_ge, Alu.subtract,
                    )
                    for j in range(NCHUNK):
                        tp = tp_p.tile(
                            [KP, G, RB], BF16, tag="tp", name=f"tpi{img}{g}{j}"
                        )
                        for b in range(G):
                            nc.tensor.transpose(
                                tp[:, b, :],
                                xg[:, b, j * KP : (j + 1) * KP],
                                id8sb[:RB, :RB],
                            )
                        srcv = tp[:, :, :].rearrange("p g (r w) -> p (g r) w", w=W)
                        a0 = DOFF + (1 + 2 * G * g) * Wp
                        dst = xsT[:, j, a0 : a0 + 2 * G * Wp].rearrange(
                            "p (r w) -> p r w", w=Wp
                        )[:, :, 1 : 1 + W]
                        nc.vector.tensor_copy(dst, srcv)

                # ---------- conv1 -> BN1+sign ----------
                hsT = hsT_p.tile(
                    [KP, NCHUNK, S_chunk], FP8, tag="hsT", name=f"hsT{img}"
                )
                border_memsets(hsT)

                def bnsign(si, j, ps, st):
                    cs, cn = st[0], st[1]
                    nc.scalar.activation(
                        hsT[:, j, DOFF + cs : DOFF + cs + cn],
                        ps[:, :cn],
                        Act.Sign,
                        bias=nt1sb[:, j : j + 1],
                        scale=1.0,
                    )

                conv(xsT, w1sb, conv1_st, bnsign)
                col_border_memset(hsT)  # after bnsign: re-zero border cols

                # ---------- conv2 -> pool -> BN2 ----------
                pr_tiles = [
                    pr_p.tile([KP, H // 2, W], F32, tag="pr", name=f"pr{img}{j}")
                    for j in range(NCHUNK)
                ]
                max_pairs = max(rg for _, rg in row_groups) // 2

                def pool1(si, j, ps, st):
                    cs, cn, r0, rg = st
                    rows = ps[:, : rg * Wp].rearrange("p (q t) -> p q t", t=2 * Wp)
                    in0 = rows[:, :, 1 : 1 + W]
                    in1 = rows[:, :, Wp + 1 : Wp + 1 + W]
                    sl = slice(r0 // 2, (r0 + rg) // 2)
                    q = rg // 2
                    prA = pr_p.tile(
                        [KP, max_pairs, W], F32, tag="prA", bufs=1,
                        name=f"prA{img}{si}{j}",
                    )
                    nc.scalar.copy(prA[:, :q, :], in0)
                    nc.vector.tensor_max(pr_tiles[j][:, sl, :], prA[:, :q, :], in1)

                conv(hsT, w2sb, conv2_st, pool1)

                for j in range(NCHUNK):
                    prf = pr_tiles[j].rearrange("p r w -> p (r w)")
                    pooled = po_p.tile([KP, PO], F32, tag="pooled", name=f"pl{img}{j}")
                    nc.vector.tensor_max(pooled, prf[:, 0::2], prf[:, 1::2])
                    nc.vector.tensor_scalar(
                        pooled, pooled, s2sb[:, j : j + 1], b2sb[:, j : j + 1],
                        Alu.mult, Alu.add,
                    )
                    pr_tiles[j] = pooled  # stash for transpose below
                if img % HB == 0:
                    onat = onat_p.tile(
                        [OB, HB, NOB, C], F32, tag="on", name=f"on{img}"
                    )
                for b in range(NOB):
                    otp = tp_p.tile(
                        [OB, NCHUNK, KP], F32, tag="tp", name=f"tpo{img}{b}"
                    )
                    for j in range(NCHUNK):
                        nc.tensor.transpose(
                            otp[:, j, :],
                            pr_tiles[j][:, OB * b : OB * (b + 1)],
                            id32sb[:, :],
                        )
                    nc.scalar.copy(
                        onat[:, img % HB, b, :],
                        otp[:, :, :].rearrange("p a b -> p (a b)"),
                    )
                if img % HB == HB - 1:
                    dst = dram_ap(
                        y_h,
                        (img - HB + 1) * PO * C,
                        [[C, OB], [OB * C, HB * NOB], [1, C]],
                    )
                    nc.sync.dma_start(
                        out=dst, in_=onat.rearrange("p a b c -> p (a b) c")
                    )

    nc.compile()
    return nc


# ---------------------------------------------------------------------------
# host-side constant prep
# ---------------------------------------------------------------------------


def _prep_consts(w1, beta1, mean1, var1, w2, beta2, mean2, var2):
    import jax
    import jax.numpy as jnp
    from jax import lax
    from concourse import mybir

    fp8np = mybir.dt.np(mybir.dt.float8e4)
    bf16np = mybir.dt.np(mybir.dt.bfloat16)

    def prep_w(w):
        ws = np.where(np.asarray(w) >= 0, np.float32(1.0), np.float32(-1.0))
        # [3,3,ci,co] -> [p, tap, j, ktile, m]; ci = ktile*128+p, co = j*128+m
        wr = ws.reshape(9, 2, KP, NCHUNK, KP).transpose(2, 0, 3, 1, 4)
        return np.ascontiguousarray(wr).astype(fp8np)

    w1p, w2p = prep_w(w1), prep_w(w2)

    cpu = jax.devices("cpu")[0]
    MAXH = 9 * C
    with jax.default_device(cpu):
        hs = jnp.arange(-MAXH, MAXH + 1, dtype=jnp.float32)
        bn1 = (hs[:, None] - jnp.asarray(mean1)[None, :]) * lax.rsqrt(
            jnp.asarray(var1) + 1e-3
        )[None, :] + jnp.asarray(beta1)[None, :]
        nonneg = np.asarray(bn1 >= 0)
        r2 = np.asarray(lax.rsqrt(jnp.asarray(var2) + 1e-3))

    assert (np.diff(nonneg.astype(np.int8), axis=0) >= 0).all(), "bn1 not monotone"
    kc = np.where(nonneg.any(0), nonneg.argmax(0), 2 * MAXH + 1) - MAXH
    # device psum holds h/2 (x=+-0.5, w=+-1): sign flips at (kc-0.5)/2
    nt1 = (-(kc.astype(np.float64) - 0.5) / 2.0).astype(np.float32)

    s2 = r2.astype(np.float32)
    b2 = (
        np.asarray(beta2, np.float64)
        - np.asarray(mean2, np.float64) * s2.astype(np.float64)
    ).astype(np.float32)

    def to_pj(a):  # [256] -> [128, 2] with c = j*128+p
        return np.ascontiguousarray(a.reshape(NCHUNK, KP).T).astype(np.float32)

    # pack everything into one [128, CONST_B] uint8 image
    cbuf = np.zeros((KP, CONST_B), dtype=np.uint8)

    def put(off, arr):
        by = np.ascontiguousarray(arr).reshape(KP, -1).view(np.uint8)
        cbuf[:, off : off + by.shape[1]] = by

    put(W1_OFF, w1p)
    put(W2_OFF, w2p)
    put(ID8_OFF, np.eye(KP, dtype=bf16np))
    put(ID32_OFF, np.eye(KP, dtype=np.float32))
    put(NT1_OFF, to_pj(nt1))
    put(S2_OFF, to_pj(s2))
    put(B2_OFF, to_pj(b2))
    return {"cb": cbuf}


# ---------------------------------------------------------------------------
# entry point
# ---------------------------------------------------------------------------

_cached = {}


def _run(inputs, trace=False):
    from concourse import bass_utils

    x = np.asarray(inputs["x"], dtype=np.float32)
    Bt, H, W, _ = x.shape  # 32, 56, 56, 256
    Bc = Bt // N_CORES

    consts = _prep_consts(
        inputs["w1"], inputs["beta1"], inputs["mean1"], inputs["var1"],
        inputs["w2"], inputs["beta2"], inputs["mean2"], inputs["var2"],
    )

    key = (Bc, H, W)
    if key not in _cached:
        _cached[key] = build_program(Bc, H, W)
    nc = _cached[key]

    in_maps = []
    for c in range(N_CORES):
        m = dict(consts)
        m["x"] = np.ascontiguousarray(x[c * Bc : (c + 1) * Bc].reshape(Bc, H * W, C))
        in_maps.append(m)

    res = bass_utils.run_bass_kernel_spmd(
        nc, in_maps, core_ids=list(range(N_CORES)), trace=trace
    )
    y = np.concatenate([r["y"] for r in res.results], axis=0)
    y = y.reshape(Bt, H // 2, W // 2, C).astype(np.float32)
    return y, res


def kernel(**inputs):
    y, _ = _run(inputs, trace=False)
    return y


# revision 20
# speedup vs baseline: 1.4302x; 1.4302x over previous
"""Trainium2 Bass kernel for a BinaryNet conv block.

Pipeline (per core, data-parallel over batch):
  sign(x) -> conv3x3(sign(w1)) -> BN1 -> sign -> conv3x3(sign(w2))
          -> maxpool2x2 -> BN2

Implementation notes:
  - Activations are +-0.5, weights +-1.0 in fp8e4 (exactly representable);
    convs run as 9 shifted-window matmuls with DoubleRow perf mode (K=256
    contraction per instruction), accumulating exactly into fp32 PSUM.
  - BN1+sign is fused into one ScalarE Sign activation against a
    host-precomputed per-channel threshold. Conv outputs are exact
    integers, so an integer cutoff k_c reproduces the reference's fp32
    sign decisions bit-exactly.
  - Spatial layout is channel-major [ci, y*(W+2)+x] with a zero border so
    the 9 taps are just constant AP offsets.
  - The bass2jax/pseudo-DMA path allows only ONE sync wait per DMA and has
    8 DMA lanes, so the kernel uses exactly 6 DMAs (1 packed consts, 4 x
    loads into DISTINCT tiles, 1 y store); no DMA destination is ever
    reused, so every DMA needs at most one semaphore wait.
"""

import os
import numpy as np

os.environ.setdefault("MYCRO_LOCAL_CACHE", "1")

N_CORES = 8
C = 256
NCHUNK = 2  # channel chunks of 128
KP = 128

# packed consts layout (bytes per partition)
W1_OFF = 0
W2_OFF = 4608
NT1_OFF = 9216  # f32 [2]
S2_OFF = 9224
B2_OFF = 9232
CONST_B = 9248


def build_program(B, H, W, psum_stretch=1024, conv_bufs=3):
    """Build the per-core Bass program. B images of HxWxC per core."""
    import concourse.bass as bass
    import concourse.bacc as bacc
    import concourse.tile as tile
    from concourse import mybir

    F32 = mybir.dt.float32
    FP8 = mybir.dt.float8e4
    BF16 = mybir.dt.bfloat16
    U8 = mybir.dt.uint8
    DR = mybir.MatmulPerfMode.DoubleRow
    Alu = mybir.AluOpType
    Act = mybir.ActivationFunctionType

    Hp, Wp = H + 2, W + 2
    S_pad = Hp * Wp
    DOFF = 32  # left zero pad inside each channel-chunk row buffer
    S_chunk = ((S_pad + DOFF + 32 + 15) // 16) * 16  # right pad >= 32
    RB = 2 * W  # transpose block = 2 image rows
    assert RB <= 128
    NB = H // 2  # transpose blocks per image
    G = 7 if NB % 7 == 0 else (2 if NB % 2 == 0 else 1)  # blocks per psum group
    NG = NB // G
    PO = (H // 2) * (W // 2)
    OB = min(112, PO)  # output transpose block (partitions)
    assert PO % OB == 0
    NOB = PO // OB

    def split_stretch(total, step):
        out, a = [], 0
        while a < total:
            out.append((a, min(step, total - a)))
            a += step
        return out

    max_rows = (psum_stretch // Wp) // 2 * 2
    row_groups = []
    r = 0
    while r < H:
        g = min(max_rows, H - r)
        row_groups.append((r, g))
        r += g
    conv2_st = [((1 + r0) * Wp, rg * Wp, r0, rg) for r0, rg in row_groups]
    conv1_st = conv2_st
    PS_COLS = psum_stretch

    nc = bacc.Bacc("TRN2", target_bir_lowering=False, debug=False)

    x_h = nc.dram_tensor("x", [B, H * W, C], F32, kind="ExternalInput")
    cb_h = nc.dram_tensor("cb", [KP, CONST_B], U8, kind="ExternalInput")
    y_h = nc.dram_tensor("y", [B, PO, C], F32, kind="ExternalOutput")

    def dram_ap(handle, offset, dims):
        return bass.AP(
            tensor=handle.ap().tensor, offset=offset, ap=[list(d) for d in dims]
        )

    with tile.TileContext(nc) as tc:
        from contextlib import ExitStack

        with ExitStack() as ctx:
            consts = ctx.enter_context(tc.tile_pool(name="consts", bufs=1))
            xnat_p = ctx.enter_context(tc.tile_pool(name="xnat", bufs=1))
            xsg_p = ctx.enter_context(tc.tile_pool(name="xsg", bufs=2))
            xsT_p = ctx.enter_context(tc.tile_pool(name="xsT", bufs=2))
            hsT_p = ctx.enter_context(tc.tile_pool(name="hsT", bufs=2))
            pr_p = ctx.enter_context(tc.tile_pool(name="prp", bufs=2))
            po_p = ctx.enter_context(tc.tile_pool(name="pop", bufs=2))
            onat_p = ctx.enter_context(tc.tile_pool(name="onat", bufs=1))
            convp = ctx.enter_context(tc.tile_pool(name="convp", bufs=conv_bufs, space="PSUM"))
            tp_p = ctx.enter_context(tc.tile_pool(name="tpp", bufs=2, space="PSUM"))

            # --- packed constants: one DMA (issued after img0's x load so
            # the input pipeline wins the DMA bandwidth race), bitcast views
            cb = consts.tile([KP, CONST_B], U8)
            cb_dma = [False]

            def load_consts():
                if not cb_dma[0]:
                    nc.sync.dma_start(out=cb, in_=cb_h.ap())
                    cb_dma[0] = True
            w1sb = cb[:, W1_OFF : W1_OFF + 4608].bitcast(FP8).rearrange(
                "p (t j k m) -> p t j k m", t=9, j=NCHUNK, k=2
            )
            w2sb = cb[:, W2_OFF : W2_OFF + 4608].bitcast(FP8).rearrange(
                "p (t j k m) -> p t j k m", t=9, j=NCHUNK, k=2
            )
            # identities built on-device (GPSIMD) so transposes don't wait
            # for the big consts DMA
            from concourse import masks

            id8sb = consts.tile([KP, KP], BF16)
            id32sb = consts.tile([KP, KP], F32)
            masks.make_identity(nc, id8sb)
            masks.make_identity(nc, id32sb)
            nt1sb = cb[:, NT1_OFF : NT1_OFF + 8].bitcast(F32)
            s2sb = cb[:, S2_OFF : S2_OFF + 8].bitcast(F32)
            b2sb = cb[:, B2_OFF : B2_OFF + 8].bitcast(F32)

            def border_memsets(buf):
                # rows 0 and H+1, left/right pads, and border cols {0, W+1} of
                # rows 1..H. Interior writes never touch these bytes, so all
                # zeroing happens up front with no WAW serialization.
                nc.vector.memset(buf[:, :, 0 : DOFF + Wp], 0.0)
                nc.vector.memset(buf[:, :, DOFF + (H + 1) * Wp : S_chunk], 0.0)
                rows = buf[:, :, DOFF + Wp : DOFF + (H + 1) * Wp].rearrange(
                    "p j (r w) -> p j r w", w=Wp
                )
                nc.vector.memset(rows[:, :, :, 0 :: (W + 1)], 0.0)

            def conv(inbuf, wsb, stretches, psum_tiles_cb):
                for si, st in enumerate(stretches):
                    cs, cn = st[0], st[1]
                    for j in range(NCHUNK):
                        ps = convp.tile([KP, PS_COLS], F32, tag="cv", name=f"cv{si}{j}")
                        for t in range(9):
                            dy, dx = t // 3, t % 3
                            off = (dy - 1) * Wp + (dx - 1)
                            lhsT = wsb[:, t, j]
                            for c0 in range(0, cn, 512):
                                n = min(512, cn - c0)
                                a = DOFF + cs + off + c0
                                rhs = inbuf[:, :, a : a + n]
                                nc.tensor.matmul(
                                    ps[:, c0 : c0 + n],
                                    lhsT,
                                    rhs,
                                    start=(t == 0),
                                    stop=(t == 8),
                                    perf_mode=DR,
                                )
                        psum_tiles_cb(si, j, ps, st)

            # output DMA groups: {0,1}, {2}, {3} for B=4; singles otherwise
            if B == 4:
                out_groups = [(0, 2), (2, 1), (3, 1)]
            else:
                out_groups = [(i, 1) for i in range(B)]
            grp_of = {}
            for g0, gn in out_groups:
                for i in range(g0, g0 + gn):
                    grp_of[i] = (g0, gn)
            onat_box = [None]
            xsT_tiles = {}
            xn_views = {}

            def get_xn(img):
                if img in xn_views:
                    return xn_views.pop(img)
                if B == 4 and img == 0:
                    xn = xnat_p.tile([RB, NB, C], F32, tag="xn0", name="xn0")
                    h1 = NB // 2
                    nc.sync.dma_start(
                        out=xn[:, :h1, :],
                        in_=dram_ap(x_h, 0, [[C, RB], [RB * C, h1], [1, C]]),
                    )
                    nc.sync.dma_start(
                        out=xn[:, h1:, :],
                        in_=dram_ap(
                            x_h, h1 * RB * C, [[C, RB], [RB * C, NB - h1], [1, C]]
                        ),
                    )
                    return xn
                if B == 4 and img == 2:
                    # one DMA covering images 2 and 3 (contiguous in DRAM)
                    xn2 = xnat_p.tile([RB, 2 * NB, C], F32, tag="xn23", name="xn23")
                    nc.sync.dma_start(
                        out=xn2,
                        in_=dram_ap(
                            x_h, 2 * H * W * C, [[C, RB], [RB * C, 2 * NB], [1, C]]
                        ),
                    )
                    xn_views[3] = xn2[:, NB:, :]
                    return xn2[:, :NB, :]
                xn = xnat_p.tile([RB, NB, C], F32, tag=f"xn{img}", name=f"xn{img}")
                nc.sync.dma_start(
                    out=xn,
                    in_=dram_ap(
                        x_h, img * H * W * C, [[C, RB], [RB * C, NB], [1, C]]
                    ),
                )
                return xn

            def prep_input(img):
                # one DMA + sign + PE transpose into channel-major fp8 layout
                xn = get_xn(img)
                xsT = xsT_p.tile(
                    [KP, NCHUNK, S_chunk], FP8, tag="xsT", name=f"xsT{img}"
                )
                border_memsets(xsT)
                for g in range(NG):
                    xg = xsg_p.tile([RB, G, C], BF16, tag="xg", name=f"xg{img}{g}")
                    nc.vector.tensor_scalar(
                        xg, xn[:, g * G : (g + 1) * G, :], 0.0, 0.5,
                        Alu.is_ge, Alu.subtract,
                    )
                    for j in range(NCHUNK):
                        tp = tp_p.tile(
                            [KP, G, RB], BF16, tag="tp", name=f"tpi{img}{g}{j}"
                        )
                        for b in range(G):
                            nc.tensor.transpose(
                                tp[:, b, :],
                                xg[:, b, j * KP : (j + 1) * KP],
                                id8sb[:RB, :RB],
                            )
                        srcv = tp[:, :, :].rearrange("p g (r w) -> p (g r) w", w=W)
                        a0 = DOFF + (1 + 2 * G * g) * Wp
                        dst = xsT[:, j, a0 : a0 + 2 * G * Wp].rearrange(
                            "p (r w) -> p r w", w=Wp
                        )[:, :, 1 : 1 + W]
                        nc.vector.tensor_copy(dst, srcv)
                xsT_tiles[img] = xsT

            def run_convs(img):
                xsT = xsT_tiles.pop(img)
                g0, gn = grp_of[img]
                # ---------- conv1 -> BN1+sign ----------
                hsT = hsT_p.tile(
                    [KP, NCHUNK, S_chunk], FP8, tag="hsT", name=f"hsT{img}"
                )
                border_memsets(hsT)

                def bnsign(si, j, ps, st):
                    cs, cn, r0, rg = st
                    dstv = hsT[:, j, DOFF + cs : DOFF + cs + cn].rearrange(
                        "p (r w) -> p r w", w=Wp
                    )[:, :, 1 : 1 + W]
                    srcv = ps[:, :cn].rearrange("p (r w) -> p r w", w=Wp)[
                        :, :, 1 : 1 + W
                    ]
                    nc.scalar.activation(
                        dstv, srcv, Act.Sign, bias=nt1sb[:, j : j + 1], scale=1.0
                    )

                conv(xsT, w1sb, conv1_st, bnsign)

                # ---------- conv2 -> pool -> BN2 -> transpose (per stretch) ---
                if img == g0:
                    onat_box[0] = onat_p.tile(
                        [OB, min(2, B), NOB, C], F32, tag="on", name=f"on{img}"
                    )
                onat = onat_box[0]
                pr_tiles = [
                    pr_p.tile([KP, H // 2, W], F32, tag="pr", name=f"pr{img}{j}")
                    for j in range(NCHUNK)
                ]
                pooled_tiles = [
                    po_p.tile([KP, PO], F32, tag="pooled", name=f"pl{img}{j}")
                    for j in range(NCHUNK)
                ]
                max_pairs = max(rg for _, rg in row_groups) // 2
                WH = W // 2

                def pool1(si, j, ps, st):
                    cs, cn, r0, rg = st
                    rows = ps[:, : rg * Wp].rearrange("p (q t) -> p q t", t=2 * Wp)
                    in0 = rows[:, :, 1 : 1 + W]
                    in1 = rows[:, :, Wp + 1 : Wp + 1 + W]
                    q0, q1 = r0 // 2, (r0 + rg) // 2
                    q = rg // 2
                    prA = pr_p.tile(
                        [KP, max_pairs, W], F32, tag="prA", bufs=1,
                        name=f"prA{img}{si}{j}",
                    )
                    nc.scalar.copy(prA[:, :q, :], in0)
                    nc.vector.tensor_max(
                        pr_tiles[j][:, q0:q1, :], prA[:, :q, :], in1
                    )
                    # pool step 2 + BN2 for this stretch's rows
                    prs = pr_tiles[j][:, q0:q1, :].rearrange("p q w -> p (q w)")
                    pv = pooled_tiles[j].rearrange("p (q w) -> p q w", w=WH)[
                        :, q0:q1, :
                    ]
                    nc.vector.tensor_max(pv, prs[:, 0::2], prs[:, 1::2])
                    nc.vector.tensor_scalar(
                        pv, pv, s2sb[:, j : j + 1], b2sb[:, j : j + 1],
                        Alu.mult, Alu.add,
                    )
                    if j == NCHUNK - 1:
                        # transpose every output block fully covered now
                        b0 = (q0 * WH + OB - 1) // OB
                        b1 = (q1 * WH) // OB
                        for b in range(b0, b1):
                            otp = tp_p.tile(
                                [OB, NCHUNK, KP], F32, tag="tp",
                                name=f"tpo{img}{b}",
                            )
                            for jj in range(NCHUNK):
                                nc.tensor.transpose(
                                    otp[:, jj, :],
                                    pooled_tiles[jj][:, OB * b : OB * (b + 1)],
                                    id32sb[:, :],
                                )
                            nc.scalar.copy(
                                onat[:, img - g0, b, :],
                                otp[:, :, :].rearrange("p a b -> p (a b)"),
                            )

                conv(hsT, w2sb, conv2_st, pool1)

                if img == g0 + gn - 1:
                    dst = dram_ap(
                        y_h, g0 * PO * C, [[C, OB], [OB * C, gn * NOB], [1, C]]
                    )
                    nc.sync.dma_start(
                        out=dst,
                        in_=onat[:, :gn, :, :].rearrange("p a b c -> p (a b) c"),
                    )

            # software-pipelined emission: input prep leads convs by one image
            prep_input(0)
            load_consts()
            for img in range(B):
                if img + 1 < B:
                    prep_input(img + 1)
                run_convs(img)

    nc.compile()
    return nc


# ---------------------------------------------------------------------------
# host-side constant prep
# ---------------------------------------------------------------------------


def _prep_consts(w1, beta1, mean1, var1, w2, beta2, mean2, var2):
    import jax
    import jax.numpy as jnp
    from jax import lax
    from concourse import mybir

    fp8np = mybir.dt.np(mybir.dt.float8e4)

    def prep_w(w):
        ws = np.where(np.asarray(w) >= 0, np.float32(1.0), np.float32(-1.0))
        # [3,3,ci,co] -> [p, tap, j, ktile, m]; ci = ktile*128+p, co = j*128+m
        wr = ws.reshape(9, 2, KP, NCHUNK, KP).transpose(2, 0, 3, 1, 4)
        return np.ascontiguousarray(wr).astype(fp8np)

    w1p, w2p = prep_w(w1), prep_w(w2)

    cpu = jax.devices("cpu")[0]
    MAXH = 9 * C
    with jax.default_device(cpu):
        hs = jnp.arange(-MAXH, MAXH + 1, dtype=jnp.float32)
        bn1 = (hs[:, None] - jnp.asarray(mean1)[None, :]) * lax.rsqrt(
            jnp.asarray(var1) + 1e-3
        )[None, :] + jnp.asarray(beta1)[None, :]
        nonneg = np.asarray(bn1 >= 0)
        r2 = np.asarray(lax.rsqrt(jnp.asarray(var2) + 1e-3))

    assert (np.diff(nonneg.astype(np.int8), axis=0) >= 0).all(), "bn1 not monotone"
    kc = np.where(nonneg.any(0), nonneg.argmax(0), 2 * MAXH + 1) - MAXH
    # device psum holds h/2 (x=+-0.5, w=+-1): sign flips at (kc-0.5)/2
    nt1 = (-(kc.astype(np.float64) - 0.5) / 2.0).astype(np.float32)

    s2 = r2.astype(np.float32)
    b2 = (
        np.asarray(beta2, np.float64)
        - np.asarray(mean2, np.float64) * s2.astype(np.float64)
    ).astype(np.float32)

    def to_pj(a):  # [256] -> [128, 2] with c = j*128+p
        return np.ascontiguousarray(a.reshape(NCHUNK, KP).T).astype(np.float32)

    # pack everything into one [128, CONST_B] uint8 image
    cbuf = np.zeros((KP, CONST_B), dtype=np.uint8)

    def put(off, arr):
        by = np.ascontiguousarray(arr).reshape(KP, -1).view(np.uint8)
        cbuf[:, off : off + by.shape[1]] = by

    put(W1_OFF, w1p)
    put(W2_OFF, w2p)
    put(NT1_OFF, to_pj(nt1))
    put(S2_OFF, to_pj(s2))
    put(B2_OFF, to_pj(b2))
    return {"cb": cbuf}


# ---------------------------------------------------------------------------
# entry point
# ---------------------------------------------------------------------------

_cached = {}


def _run(inputs, trace=False):
    from concourse import bass_utils

    x = np.asarray(inputs["x"], dtype=np.float32)
    Bt, H, W, _ = x.shape  # 32, 56, 56, 256
    Bc = Bt // N_CORES

    consts = _prep_consts(
        inputs["w1"], inputs["beta1"], inputs["mean1"], inputs["var1"],
        inputs["w2"], inputs["beta2"], inputs["mean2"], inputs["var2"],
    )

    key = (Bc, H, W)
    if key not in _cached:
        _cached[key] = build_program(Bc, H, W)
    nc = _cached[key]

    in_maps = []
    for c in range(N_CORES):
        m = dict(consts)
        m["x"] = np.ascontiguousarray(x[c * Bc : (c + 1) * Bc].reshape(Bc, H * W, C))
        in_maps.append(m)

    res = bass_utils.run_bass_kernel_spmd(
        nc, in_maps, core_ids=list(range(N_CORES)), trace=trace
    )
    y = np.concatenate([r["y"] for r in res.results], axis=0)
    y = y.reshape(Bt, H // 2, W // 2, C).astype(np.float32)
    return y, res


def kernel(**inputs):
    y, _ = _run(inputs, trace=False)
    return y
